# revision 5
# baseline (speedup 1.0000x reference)
"""Trainium2 Bass kernel for chunked-local GQA attention (nn_Attention_12266426597578).

Full-input contract: kernel(**inputs) takes the unsharded numpy inputs
(x [1,4096,4096], wq [4096,4096], wk [1024,4096], wv [1024,4096],
wo [4096,4096]) and returns the full output [1,4096,4096].

Sharding (8 cores): 2 chunk-groups (2 attention chunks of 1024 tokens each)
x 4 head-groups (8 q-heads / 2 kv-heads each). Each core computes a partial
y^T [D, 2048] for its token range; the host sums the 4 head-group partials
per token range and concatenates.

Device-side layout: everything runs in the "transposed" layout (feature dim
on SBUF partitions, tokens on the free axis) so that QKV projections,
RoPE (a 128x128 pair-rotation matmul), RMSNorm reductions (ones-vector
matmuls), scores^T, softmax denominators and PV all map onto the PE array
with no on-device transposition of x or the weights (the host pre-transposes
them instead).

v2 notes (trace-driven): the projection tiles live in bf16 end-to-end (no
fp32 shadow, no gpsimd cast on the RoPE critical path), squares run on the
vector engine, and every 1/x / 1/sqrt(x) row is computed with the DVE
reciprocal_approx_fast custom op on replicated tiles instead of the ACT
Ln/Exp table dance (ACT keeps only Exp + Sqrt + Copy; no per-head table
reloads). Post-processing of each projection group is deferred one group so
the PE stream stays dense and the HAM clock gate stays warm.
"""

import sys

sys.path.insert(0, "/opt/trn_rl_repo")

import numpy as np
import ml_dtypes
from contextlib import ExitStack

import concourse.bass as bass  # noqa: F401
import concourse.tile as tile
from concourse import bacc, mybir
from concourse.bass_utils import run_bass_kernel_spmd

# Problem constants (hardcoded per contract)
S, D = 4096, 4096
H, KVH, HD = 32, 8, 128
CHUNK = 1024
EPS = 1e-5
THETA = 500000.0
ISQ_HD = 1.0 / np.sqrt(np.float32(HD))

# Sharding
NCORES = 8
CG = 2  # chunk groups (token split)
HG = 4  # head groups
TOK = S // CG  # 2048 tokens per core
NCH = TOK // CHUNK  # 2 chunks per core
QH = H // HG  # 8 q heads per core
KH = KVH // HG  # 2 kv heads per core
QO = QH * HD  # 1024
KO = KH * HD  # 256
QKVO = QO + 2 * KO  # 1536
NOT = QKVO // 128  # 12 o-tiles: 0..1 k, 2..3 v, 4..11 q
NB = CHUNK // 128  # 8 blocks of 128 tokens per chunk

f32 = mybir.dt.float32
f32r = mybir.dt.float32r
bf16 = mybir.dt.bfloat16

# knobs
WBUFS = 8
XT_EXTRA = 2
GW = 2  # o-tiles per PSUM group in phase 1
_CACHE = {}

SDT = bf16
SNP = ml_dtypes.bfloat16


def _emit_chunk(ctx, tc, ci, io, pools):
    nc = tc.nc
    (xT, wqkvT, woT, yT) = io["dram"]
    consts = io["consts"]
    sb, ps = pools["sb"], pools["ps"]
    s0 = ci * CHUNK

    onesb_t = consts["onesb"]
    beps_t = consts["beps"]
    rmat_t = consts["rmat"]
    identb_t = consts["identb"]
    mask_t = consts["mask"]
    costab_t = consts["costab"]
    sintab_t = consts["sintab"]
    onesr_t = consts["onesr"]

    # ---------------- Phase 1: QKV projection (+RoPE +RMSNorm) ----------------
    # o-tile order (host-side wqkv concat order): k0 k1 v0 v1 q0..q7 so that
    # k/v post-processing and the first attention heads start while later q
    # projections are still running.
    qk_tiles = []
    for ot in range(NOT):
        t = sb.tile([128, CHUNK], bf16, name=f"qkv_{ci}_{ot}", tag="qk", bufs=6)
        qk_tiles.append(t)

    rq_rows = [None] * QH
    rk_cols = [None] * KH
    v_tok = [None] * KH
    fin_tiles = [None] * NOT
    pending = []

    def post(ot):
        """RoPE + RMSNorm (q/k) or transpose (v); all-bf16 tiles feed the
        attention matmuls. 1/sqrt rows use DVE reciprocal_approx_fast + ACT
        Sqrt (no Ln/Exp -> no ACT table thrash)."""
        raw = qk_tiles[ot]
        if 2 <= ot < 4:  # v: bf16 PE transpose per 128-block
            kvh = ot - 2
            vt = sb.tile([128, CHUNK], bf16, name=f"v_{ci}_{kvh}", tag="v", bufs=2)
            for b in range(NB):
                tp = ps.tile([128, 128], bf16, name=f"vtp_{ci}_{kvh}_{b}", tag="misc", bufs=2)
                nc.tensor.matmul(
                    tp[:],
                    lhsT=raw[:, b * 128 : (b + 1) * 128],
                    rhs=identb_t[:],
                    is_transpose=True,
                )
                nc.vector.tensor_copy(vt[:, b * 128 : (b + 1) * 128], tp[:])
            v_tok[kvh] = vt
            return

        is_q = ot >= 4
        # sumsq over head-dim partitions (RoPE-invariant -> use raw tile);
        # squares on DVE (bf16 2x mode), keeps Square off the ACT tables.
        sqh = []
        for sh in range(2):
            t = sb.tile([128, 512], bf16, name=f"sq_{ci}_{ot}_{sh}", tag="sq", bufs=4)
            nc.vector.tensor_tensor(
                t[:], raw[:, sh * 512 : (sh + 1) * 512],
                raw[:, sh * 512 : (sh + 1) * 512], mybir.AluOpType.mult,
            )
            sqh.append(t)
        if is_q:
            # row 1/rms: mean+eps via ACT Copy (scale/bias), reciprocal on the
            # row with the fast DVE approx, then ACT Sqrt -> bf16 row.
            h = ot - 4
            ssq = ps.tile([1, 512], f32, name=f"ssqa_{ci}_{ot}", tag="misc", bufs=2)
            ssq2 = ps.tile([1, 512], f32, name=f"ssqb_{ci}_{ot}", tag="misc", bufs=2)
            nc.tensor.matmul(ssq[:], lhsT=onesb_t[:], rhs=sqh[0][:])
            nc.tensor.matmul(ssq2[:], lhsT=onesb_t[:], rhs=sqh[1][:])
            rows = sb.tile([1, CHUNK], bf16, name=f"rqr_{ci}_{ot}", tag="rowq", bufs=3)
            nc.scalar.activation(
                rows[:, 0:512], ssq[:], mybir.ActivationFunctionType.Sqrt,
                bias=beps_t[0:1, :], scale=float(1.0 / HD),
            )
            nc.scalar.activation(
                rows[:, 512:1024], ssq2[:], mybir.ActivationFunctionType.Sqrt,
                bias=beps_t[0:1, :], scale=float(1.0 / HD),
            )
            rq_rows[h] = rows
        else:
            kvh = ot
            ssqc = ps.tile([128, NB], f32, name=f"ssqk_{ci}_{kvh}", tag="misc", bufs=2)
            for b in range(NB):
                nc.tensor.matmul(
                    ssqc[:, b : b + 1],
                    lhsT=sqh[b // 4][:, (b % 4) * 128 : (b % 4 + 1) * 128],
                    rhs=onesb_t[:],
                )
            rootc = sb.tile([128, NB], f32, name=f"rootk_{ci}_{kvh}", tag="rk", bufs=10)
            nc.scalar.activation(
                rootc[:], ssqc[:], mybir.ActivationFunctionType.Sqrt,
                bias=beps_t[:, :], scale=float(1.0 / HD),
            )
            rkinv = sb.tile([128, NB], f32, name=f"rk_{ci}_{kvh}", tag="rk", bufs=10)
            nc.vector.reciprocal(rkinv[:], rootc[:])
            rk_cols[kvh] = rkinv

        # RoPE (all bf16): fin <- raw*cos + (R @ raw)*sin [*1/rms for q]
        fin = sb.tile([128, CHUNK], bf16, name=f"fin_{ci}_{ot}", tag="qb", bufs=11)
        for sh in range(2):
            rot = ps.tile([128, 512], f32, name=f"rot_{ci}_{ot}_{sh}", tag="mm", bufs=6)
            nc.tensor.matmul(
                rot[:], lhsT=rmat_t[:], rhs=raw[:, sh * 512 : (sh + 1) * 512]
            )
            nc.vector.tensor_tensor(
                rot[:], rot[:], sintab_t[:, s0 + sh * 512 : s0 + (sh + 1) * 512],
                mybir.AluOpType.mult,
            )
            nc.vector.tensor_tensor(
                raw[:, sh * 512 : (sh + 1) * 512],
                raw[:, sh * 512 : (sh + 1) * 512],
                costab_t[:, s0 + sh * 512 : s0 + (sh + 1) * 512],
                mybir.AluOpType.mult,
            )
            if is_q:
                nc.vector.tensor_tensor(
                    raw[:, sh * 512 : (sh + 1) * 512],
                    raw[:, sh * 512 : (sh + 1) * 512],
                    rot[:],
                    mybir.AluOpType.add,
                )
            else:
                nc.vector.tensor_tensor(
                    fin[:, sh * 512 : (sh + 1) * 512],
                    raw[:, sh * 512 : (sh + 1) * 512],
                    rot[:],
                    mybir.AluOpType.add,
                )
        if is_q:
            def fin_q(ot=ot, raw=raw, fin=fin):
                rows = rq_rows[ot - 4]
                for sh in range(2):
                    rep = ps.tile([128, 512], f32, name=f"repq_{ci}_{ot}_{sh}", tag="mm", bufs=6)
                    nc.tensor.matmul(
                        rep[:], lhsT=onesr_t[:],
                        rhs=rows[:, sh * 512 : (sh + 1) * 512],
                    )
                    rinv = sb.tile([128, 512], f32, name=f"rq_{ci}_{ot}_{sh}", tag="sq", bufs=4)
                    nc.vector.reciprocal_approx_fast(rinv[:], rep[:])
                    nc.vector.tensor_tensor(
                        fin[:, sh * 512 : (sh + 1) * 512],
                        raw[:, sh * 512 : (sh + 1) * 512],
                        rinv[:],
                        mybir.AluOpType.mult,
                    )
            pending.append(fin_q)
        fin_tiles[ot] = fin

    DPT = D // 128  # d-tiles
    xts = io["xts"].setdefault(ci, [None] * DPT)

    def xtile(dk):
        if xts[dk] is None:
            xt = sb.tile([128, CHUNK], SDT, name=f"xt_{ci}_{dk}", tag="xt",
                         bufs=DPT + XT_EXTRA)
            nc.sync.dma_start(
                xt[:],
                xT[dk * 128 : (dk + 1) * 128, s0 : s0 + CHUNK],
            )
            xts[dk] = xt
        return xts[dk]

    def prefetch_next():
        nxts = io["xts"].setdefault(ci + 1, [None] * DPT)
        ns0 = (ci + 1) * CHUNK
        for dk in range(8):
            xt = sb.tile([128, CHUNK], SDT, name=f"xt_{ci+1}_{dk}", tag="xt",
                         bufs=DPT + XT_EXTRA)
            nc.sync.dma_start(
                xt[:],
                xT[dk * 128 : (dk + 1) * 128, ns0 : ns0 + CHUNK],
            )
            nxts[dk] = xt
    io["prefetch"] = prefetch_next if ci + 1 < NCH else None

    post_q = []  # deferred post() work, flushed one og-group later

    def flush_posts():
        work = post_q[:]
        post_q.clear()
        flushq = pending[:]
        pending.clear()
        for ot in work:
            post(ot)
        for f in flushq:
            f()

    DKB = 8  # d-tiles per weight DMA (batched to offload the sync engine)
    for og in range(NOT // GW):
        accs = [
            ps.tile([128, 512], f32, name=f"pj_{ci}_{og}_{i}", tag="mm", bufs=6)
            for i in range(2 * GW)
        ]
        wts = [None] * (DPT // DKB)
        for dk in range(DPT):
            if dk % DKB == 0:
                wt = sb.tile([128, 128 * GW * DKB], SDT,
                             name=f"wt_{ci}_{og}_{dk}", tag="w", bufs=3)
                nc.sync.dma_start(
                    wt[:], wqkvT[og, dk // DKB], 
                )
                wts[dk // DKB] = wt
            wt = wts[dk // DKB]
            wo_ = (dk % DKB) * 128 * GW
            if dk == 2 and og > 0:
                # defer the previous group's post-processing until the PE has
                # fresh projection work queued, so PE never waits on it
                flush_posts()
            for j in range(GW):
                for sh in range(2):
                    nc.tensor.matmul(
                        accs[2 * j + sh][:],
                        lhsT=wt[:, wo_ + j * 128 : wo_ + (j + 1) * 128],
                        rhs=xtile(dk)[:, sh * 512 : (sh + 1) * 512],
                        start=(dk == 0),
                        stop=(dk == DPT - 1),
                    )
        for j in range(GW):
            ot = og * GW + j
            for sh in range(2):
                dst = qk_tiles[ot][:, sh * 512 : (sh + 1) * 512]
                if (2 * j + sh) % 2 == 0:
                    nc.scalar.copy(dst, accs[2 * j + sh][:])
                else:
                    nc.vector.tensor_copy(dst, accs[2 * j + sh][:])
            post_q.append(ot)

    # ---------------- Phase 2: attention (block-causal within chunk) ----------------
    # the last projection group's post-processing (flush_posts below) hides
    # under head 0's score matmuls instead of stalling the PE stream
    attn_tiles = []
    hpending = []
    rks_t = [None] * KH
    for kvh in range(KH):
        rks = sb.tile([128, NB], f32, name=f"rks_{ci}_{kvh}", tag="rk", bufs=10)
        nc.vector.tensor_scalar_mul(rks[:], rk_cols[kvh][:], float(ISQ_HD))
        rks_t[kvh] = rks
    for kvh in range(KH):
        kf = fin_tiles[kvh]
        vt = v_tok[kvh]
        rks = rks_t[kvh]
        for h4 in range(4):
            h = kvh * 4 + h4
            if h == 1:
                flush_posts()
            qf = fin_tiles[4 + h]
            pvA = ps.tile([128, 512], f32, name=f"pvA_{ci}_{h}", tag="mm", bufs=6)
            pvB = ps.tile([128, 512], f32, name=f"pvB_{ci}_{h}", tag="mm", bufs=6)
            smA = ps.tile([1, 512], f32, name=f"smA_{ci}_{h}", tag="misc", bufs=2)
            smB = ps.tile([1, 512], f32, name=f"smB_{ci}_{h}", tag="misc", bufs=2)
            flush = hpending[:]
            hpending.clear()
            pend1 = pending[:]
            pending.clear()
            for j in range(NB):
                w = CHUNK - j * 128
                lenA = 512 - j * 128 if j < 4 else 0
                pt = sb.tile([128, CHUNK], bf16, name=f"pt_{ci}_{h}_{j}", tag="pt", bufs=3)
                if lenA > 0:
                    sc = ps.tile([128, 512], f32, name=f"scA_{ci}_{h}_{j}", tag="mm", bufs=6)
                    nc.tensor.matmul(
                        sc[:, 0:lenA],
                        lhsT=kf[:, j * 128 : (j + 1) * 128],
                        rhs=qf[:, j * 128 : 512],
                    )
                    nc.scalar.activation(
                        pt[:, 0:lenA], sc[:, 0:lenA],
                        mybir.ActivationFunctionType.Exp,
                        scale=rks[:, j : j + 1],
                    )
                sc2 = ps.tile([128, 512], f32, name=f"scB_{ci}_{h}_{j}", tag="mm", bufs=6)
                b0 = max(512, j * 128)
                nc.tensor.matmul(
                    sc2[:, 0 : CHUNK - b0],
                    lhsT=kf[:, j * 128 : (j + 1) * 128],
                    rhs=qf[:, b0:CHUNK],
                )
                nc.scalar.activation(
                    pt[:, b0 - j * 128 : w], sc2[:, 0 : CHUNK - b0],
                    mybir.ActivationFunctionType.Exp,
                    scale=rks[:, j : j + 1],
                )
                nc.vector.tensor_tensor(
                    pt[:, 0:128], pt[:, 0:128], mask_t[:], mybir.AluOpType.mult
                )
                if lenA > 0:
                    nc.tensor.matmul(
                        smA[:, j * 128 : 512], lhsT=onesb_t[:], rhs=pt[:, 0:lenA],
                        start=(j == 0), stop=(j == 3),
                    )
                nc.tensor.matmul(
                    smB[:, b0 - 512 : 512], lhsT=onesb_t[:],
                    rhs=pt[:, b0 - j * 128 : w],
                    start=(j == 0), stop=(j == NB - 1),
                )
                if lenA > 0:
                    nc.tensor.matmul(
                        pvA[:, j * 128 : 512],
                        lhsT=vt[:, j * 128 : (j + 1) * 128],
                        rhs=pt[:, 0:lenA],
                        start=(j == 0), stop=(j == 3),
                    )
                nc.tensor.matmul(
                    pvB[:, b0 - 512 : 512],
                    lhsT=vt[:, j * 128 : (j + 1) * 128],
                    rhs=pt[:, b0 - j * 128 : w],
                    start=(j == 0), stop=(j == NB - 1),
                )
            for f in flush + pend1:
                f()
            # denominators: copy the sum rows out of PSUM (ACT Copy -> bf16),
            # replicate across partitions on PE, reciprocal on DVE; the
            # normalize is deferred one head so PE never stalls on this chain.
            srow = sb.tile([1, CHUNK], bf16, name=f"srow_{ci}_{h}", tag="rowq", bufs=3)
            nc.vector.tensor_copy(srow[:, 0:512], smA[:])
            nc.vector.tensor_copy(srow[:, 512:1024], smB[:])
            at = sb.tile([128, CHUNK], SDT, name=f"attn_{ci}_{h}", tag="attn", bufs=8)

            def fin_head(h=h, pvA=pvA, pvB=pvB, srow=srow, at=at):
                for sh, pv in ((0, pvA), (1, pvB)):
                    rep = ps.tile([128, 512], f32, name=f"reps_{ci}_{h}_{sh}", tag="mm", bufs=6)
                    nc.tensor.matmul(
                        rep[:], lhsT=onesr_t[:],
                        rhs=srow[:, sh * 512 : (sh + 1) * 512],
                    )
                    rinv = sb.tile([128, 512], f32, name=f"rinv_{ci}_{h}_{sh}", tag="sq", bufs=4)
                    nc.vector.reciprocal_approx_fast(rinv[:], rep[:])
                    nc.vector.tensor_tensor(
                        at[:, sh * 512 : (sh + 1) * 512],
                        pv[:],
                        rinv[:],
                        mybir.AluOpType.mult,
                    )
            hpending.append(fin_head)
            attn_tiles.append(at)

    for f in hpending:
        f()
    hpending.clear()

    # ---------------- Phase 3: output projection ----------------
    # y^T[dd, s] = sum_h woT[o_h, dd].T @ attn^T[o_h, s]; one 128-row dd tile
    # per group (2 PSUM banks) for deep cross-group pipelining.
    for dd in range(32):
        if dd == 16 and io.get("prefetch") is not None:
            io["prefetch"]()
        yps = [
            ps.tile([128, 512], f32, name=f"y_{ci}_{dd}_{i}", tag="mm", bufs=6)
            for i in range(2)
        ]
        wob = sb.tile([128, QO], SDT, name=f"wo_{ci}_{dd}", tag="wo", bufs=4)
        nc.sync.dma_start(wob[:], woT[dd])
        for h in range(QH):
            for sh in range(2):
                nc.tensor.matmul(
                    yps[sh][:],
                    lhsT=wob[:, h * 128 : (h + 1) * 128],
                    rhs=attn_tiles[h][:, sh * 512 : (sh + 1) * 512],
                    start=(h == 0), stop=(h == QH - 1),
                )
        ysb = sb.tile([128, CHUNK], bf16, name=f"ysb_{ci}_{dd}", tag="y", bufs=2)
        nc.vector.tensor_copy(ysb[:, 0:512], yps[0][:])
        nc.scalar.copy(ysb[:, 512:1024], yps[1][:])
        nc.sync.dma_start(
            yT[dd * 128 : (dd + 1) * 128, s0 : s0 + CHUNK], ysb[:]
        )


def _build_program():
    nc = bacc.Bacc(
        "TRN2",
        target_bir_lowering=False,
        debug=False,
        enable_asserts=False,
        num_devices=NCORES,
    )
    xT = nc.dram_tensor("xT", [D, TOK], SDT, kind="ExternalInput").ap()
    # batched-DMA layout: [og, dkb, p, (dk in batch) x (GW*128 cols)]
    wqkvT = nc.dram_tensor(
        "wqkvT", [QKVO // 256, 4, 128, 2048], SDT, kind="ExternalInput"
    ).ap()
    # per-dd-block tiled layout: woT[dd, p, h*128+c] = wo[dd*128+c, hg*QO + h*128+p]
    woT = nc.dram_tensor("woT", [D // 128, 128, QO], SDT, kind="ExternalInput").ap()
    costab = nc.dram_tensor("costab", [HD, TOK], f32, kind="ExternalInput").ap()
    sintab = nc.dram_tensor("sintab", [HD, TOK], f32, kind="ExternalInput").ap()
    rmat = nc.dram_tensor("rmat", [128, 128], bf16, kind="ExternalInput").ap()
    identb = nc.dram_tensor("identb", [128, 128], bf16, kind="ExternalInput").ap()
    mask = nc.dram_tensor("mask", [128, 128], bf16, kind="ExternalInput").ap()
    onesb = nc.dram_tensor("onesb", [128, 1], bf16, kind="ExternalInput").ap()
    beps = nc.dram_tensor("beps", [128, 1], f32, kind="ExternalInput").ap()
    onesr = nc.dram_tensor("onesr", [1, 128], bf16, kind="ExternalInput").ap()
    yT = nc.dram_tensor("yT", [D, TOK], bf16, kind="ExternalOutput").ap()

    with tile.TileContext(nc) as tc, ExitStack() as ctx:
        ctx.enter_context(nc.allow_low_precision(reason="bf16 attention operands"))
        sb = ctx.enter_context(tc.tile_pool(name="sb", bufs=1))
        ps = ctx.enter_context(tc.tile_pool(name="ps", bufs=1, space="PSUM"))
        cp = ctx.enter_context(tc.tile_pool(name="cp", bufs=1))

        consts = {}
        for nm, ap_, shape, dt_ in (
            ("costab", costab, [HD, TOK], f32),
            ("sintab", sintab, [HD, TOK], f32),
            ("rmat", rmat, [128, 128], bf16),
            ("identb", identb, [128, 128], bf16),
            ("mask", mask, [128, 128], bf16),
            ("onesb", onesb, [128, 1], bf16),
            ("beps", beps, [128, 1], f32),
            ("onesr", onesr, [1, 128], bf16),
        ):
            t = cp.tile(shape, dt_, name=f"c_{nm}")
            nc.gpsimd.dma_start(t[:], ap_[:])
            consts[nm] = t

        io = {"dram": (xT, wqkvT, woT, yT), "consts": consts, "xts": {}}
        pools = {"sb": sb, "ps": ps}
        for ci in range(NCH):
            _emit_chunk(ctx, tc, ci, io, pools)

    nc.compile()
    return nc


def _host_inputs(x, wq, wk, wv, wo):
    xf = np.ascontiguousarray(x.reshape(S, D).T.astype(SNP))  # [D, S]
    half = HD // 2
    inv_freq = (1.0 / (THETA ** (np.arange(0, half, dtype=np.float32) / half))).astype(
        np.float32
    )
    ang = np.arange(S, dtype=np.float32)[:, None] * inv_freq[None, :]
    cos = np.cos(ang).astype(np.float32)
    sin = np.sin(ang).astype(np.float32)
    costab = np.empty((HD, S), np.float32)
    sintab = np.empty((HD, S), np.float32)
    costab[0::2, :] = cos.T
    costab[1::2, :] = cos.T
    sintab[0::2, :] = sin.T
    sintab[1::2, :] = sin.T

    rmat = np.zeros((128, 128), ml_dtypes.bfloat16)
    for i in range(64):
        rmat[2 * i + 1, 2 * i] = -1.0
        rmat[2 * i, 2 * i + 1] = 1.0
    identb = np.eye(128, dtype=ml_dtypes.bfloat16)
    mask = np.triu(np.ones((128, 128), np.float32)).astype(ml_dtypes.bfloat16)
    onesb = np.ones((128, 1), ml_dtypes.bfloat16)
    beps = np.full((128, 1), EPS, np.float32)
    onesr = np.ones((1, 128), ml_dtypes.bfloat16)

    xT_cg = [np.ascontiguousarray(xf[:, cg * TOK : (cg + 1) * TOK]) for cg in range(CG)]
    cos_cg = [np.ascontiguousarray(costab[:, cg * TOK : (cg + 1) * TOK]) for cg in range(CG)]
    sin_cg = [np.ascontiguousarray(sintab[:, cg * TOK : (cg + 1) * TOK]) for cg in range(CG)]
    wqkvT_hg = []
    woT_hg = []
    for hg in range(HG):
        wq_c = wq[hg * QO : (hg + 1) * QO]
        wk_c = wk[hg * KO : (hg + 1) * KO]
        wv_c = wv[hg * KO : (hg + 1) * KO]
        # column order on device: [k, v, q]; batched-DMA tiling:
        # [og, dkb, p, dk_in_batch*256 + c] = W^T[dkb*1024 + dk*128 + p, og*256 + c]
        wflat = np.concatenate([wk_c, wv_c, wq_c], 0).T.astype(SNP)  # [D, QKVO]
        wt4 = wflat.reshape(4, 8, 128, QKVO // 256, 256)  # [dkb, dk, p, og, c]
        wqkvT_hg.append(np.ascontiguousarray(wt4.transpose(3, 0, 2, 1, 4).reshape(
            QKVO // 256, 4, 128, 2048)))
        wo_c = wo[:, hg * QO : (hg + 1) * QO]  # [D, QO]
        woH = wo_c.reshape(D // 128, 128, QH, 128).transpose(0, 3, 2, 1)  # [dd, p, hb, c]
        woT_hg.append(np.ascontiguousarray(woH.reshape(D // 128, 128, QO).astype(SNP)))

    in_maps = []
    for c in range(NCORES):
        cg, hg = c // HG, c % HG
        in_maps.append(
            {
                "xT": xT_cg[cg],
                "wqkvT": wqkvT_hg[hg],
                "woT": woT_hg[hg],
                "costab": cos_cg[cg],
                "sintab": sin_cg[cg],
                "rmat": rmat,
                "identb": identb,
                "mask": mask,
                "onesb": onesb,
                "beps": beps,
                "onesr": onesr,
            }
        )
    return in_maps


def _assemble(results):
    y = np.empty((S, D), np.float32)
    for cg in range(CG):
        acc = results[cg * HG]["yT"].astype(np.float32)
        for hg in range(1, HG):
            acc = acc + results[cg * HG + hg]["yT"].astype(np.float32)
        y[cg * TOK : (cg + 1) * TOK, :] = acc.T
    return y.reshape(1, S, D)


def kernel(x, wq, wk, wv, wo, **_kw):
    x = np.asarray(x, np.float32)
    wq = np.asarray(wq, np.float32)
    wk = np.asarray(wk, np.float32)
    wv = np.asarray(wv, np.float32)
    wo = np.asarray(wo, np.float32)

    if "nc" not in _CACHE:
        _CACHE["nc"] = _build_program()
    nc = _CACHE["nc"]
    in_maps = _host_inputs(x, wq, wk, wv, wo)
    res = run_bass_kernel_spmd(nc, in_maps, core_ids=list(range(NCORES)))
    _CACHE["last_result"] = res
    return _assemble(res.results)


def run_traced(x, wq, wk, wv, wo):
    """Like kernel() but with NTFF tracing; returns (out, BassKernelResults)."""
    if "nc" not in _CACHE:
        _CACHE["nc"] = _build_program()
    nc = _CACHE["nc"]
    in_maps = _host_inputs(
        np.asarray(x, np.float32), np.asarray(wq, np.float32),
        np.asarray(wk, np.float32), np.asarray(wv, np.float32),
        np.asarray(wo, np.float32),
    )
    res = run_bass_kernel_spmd(nc, in_maps, core_ids=list(range(NCORES)), trace=True)
    return _assemble(res.results), res


# revision 6
# speedup vs baseline: 1.0093x; 1.0093x over previous
"""Trainium2 Bass kernel for chunked-local GQA attention (nn_Attention_12266426597578).

Full-input contract: kernel(**inputs) takes the unsharded numpy inputs
(x [1,4096,4096], wq [4096,4096], wk [1024,4096], wv [1024,4096],
wo [4096,4096]) and returns the full output [1,4096,4096].

Sharding (8 cores): 2 chunk-groups (2 attention chunks of 1024 tokens each)
x 4 head-groups (8 q-heads / 2 kv-heads each). Each core computes a partial
y^T [D, 2048] for its token range; the host sums the 4 head-group partials
per token range and concatenates.

Device-side layout: everything runs in the "transposed" layout (feature dim
on SBUF partitions, tokens on the free axis) so that QKV projections,
RoPE (a 128x128 pair-rotation matmul), RMSNorm reductions (ones-vector
matmuls), scores^T, softmax denominators and PV all map onto the PE array
with no on-device transposition of x or the weights (the host pre-transposes
them instead).

v2 notes (trace-driven): the projection tiles live in bf16 end-to-end (no
fp32 shadow, no gpsimd cast on the RoPE critical path), squares run on the
vector engine, and every 1/x / 1/sqrt(x) row is computed with the DVE
reciprocal_approx_fast custom op on replicated tiles instead of the ACT
Ln/Exp table dance (ACT keeps only Exp + Sqrt + Copy; no per-head table
reloads). Post-processing of each projection group is deferred one group so
the PE stream stays dense and the HAM clock gate stays warm.
"""

import sys

sys.path.insert(0, "/opt/trn_rl_repo")

import numpy as np
import ml_dtypes
from contextlib import ExitStack

import concourse.bass as bass  # noqa: F401
import concourse.tile as tile
from concourse import bacc, mybir
from concourse.bass_utils import run_bass_kernel_spmd

# Problem constants (hardcoded per contract)
S, D = 4096, 4096
H, KVH, HD = 32, 8, 128
CHUNK = 1024
EPS = 1e-5
THETA = 500000.0
ISQ_HD = 1.0 / np.sqrt(np.float32(HD))

# Sharding
NCORES = 8
CG = 2  # chunk groups (token split)
HG = 4  # head groups
TOK = S // CG  # 2048 tokens per core
NCH = TOK // CHUNK  # 2 chunks per core
QH = H // HG  # 8 q heads per core
KH = KVH // HG  # 2 kv heads per core
QO = QH * HD  # 1024
KO = KH * HD  # 256
QKVO = QO + 2 * KO  # 1536
NOT = QKVO // 128  # 12 o-tiles: 0..1 k, 2..3 v, 4..11 q
NB = CHUNK // 128  # 8 blocks of 128 tokens per chunk

f32 = mybir.dt.float32
f32r = mybir.dt.float32r
bf16 = mybir.dt.bfloat16

# knobs
WBUFS = 8
XT_EXTRA = 2
GW = 2  # o-tiles per PSUM group in phase 1
_CACHE = {}

SDT = bf16
SNP = ml_dtypes.bfloat16


def _emit_chunk(ctx, tc, ci, io, pools):
    nc = tc.nc
    (xT, wqkvT, woT, yT) = io["dram"]
    consts = io["consts"]
    sb, ps = pools["sb"], pools["ps"]
    s0 = ci * CHUNK

    onesb_t = consts["onesb"]
    beps_t = consts["beps"]
    rmat_t = consts["rmat"]
    identb_t = consts["identb"]
    mask_t = consts["mask"]
    costab_t = consts["costab"]
    sintab_t = consts["sintab"]
    onesr_t = consts["onesr"]

    # ---------------- Phase 1: QKV projection (+RoPE +RMSNorm) ----------------
    # o-tile order (host-side wqkv concat order): k0 k1 v0 v1 q0..q7 so that
    # k/v post-processing and the first attention heads start while later q
    # projections are still running.
    qk_tiles = []
    for ot in range(NOT):
        t = sb.tile([128, CHUNK], bf16, name=f"qkv_{ci}_{ot}", tag="qk", bufs=6)
        qk_tiles.append(t)

    rq_rows = [None] * QH
    rk_cols = [None] * KH
    v_tok = [None] * KH
    fin_tiles = [None] * NOT
    pending = []

    def post(ot):
        """RoPE + RMSNorm (q/k) or transpose (v); all-bf16 tiles feed the
        attention matmuls. 1/sqrt rows use DVE reciprocal_approx_fast + ACT
        Sqrt (no Ln/Exp -> no ACT table thrash)."""
        raw = qk_tiles[ot]
        if 2 <= ot < 4:  # v: bf16 PE transpose per 128-block
            kvh = ot - 2
            vt = sb.tile([128, CHUNK], bf16, name=f"v_{ci}_{kvh}", tag="v", bufs=2)
            for b in range(NB):
                tp = ps.tile([128, 128], bf16, name=f"vtp_{ci}_{kvh}_{b}", tag="misc", bufs=2)
                nc.tensor.matmul(
                    tp[:],
                    lhsT=raw[:, b * 128 : (b + 1) * 128],
                    rhs=identb_t[:],
                    is_transpose=True,
                )
                nc.vector.tensor_copy(vt[:, b * 128 : (b + 1) * 128], tp[:])
            v_tok[kvh] = vt
            return

        is_q = ot >= 4
        # sumsq over head-dim partitions (RoPE-invariant -> use raw tile);
        # squares on DVE (bf16 2x mode), keeps Square off the ACT tables.
        sqh = []
        for sh in range(2):
            t = sb.tile([128, 512], bf16, name=f"sq_{ci}_{ot}_{sh}", tag="sq", bufs=4)
            nc.vector.tensor_tensor(
                t[:], raw[:, sh * 512 : (sh + 1) * 512],
                raw[:, sh * 512 : (sh + 1) * 512], mybir.AluOpType.mult,
            )
            sqh.append(t)
        if is_q:
            # row 1/rms: mean+eps via ACT Copy (scale/bias), reciprocal on the
            # row with the fast DVE approx, then ACT Sqrt -> bf16 row.
            h = ot - 4
            ssq = ps.tile([1, 512], f32, name=f"ssqa_{ci}_{ot}", tag="misc", bufs=2)
            ssq2 = ps.tile([1, 512], f32, name=f"ssqb_{ci}_{ot}", tag="misc", bufs=2)
            nc.tensor.matmul(ssq[:], lhsT=onesb_t[:], rhs=sqh[0][:])
            nc.tensor.matmul(ssq2[:], lhsT=onesb_t[:], rhs=sqh[1][:])
            rows = sb.tile([1, CHUNK], bf16, name=f"rqr_{ci}_{ot}", tag="rowq", bufs=3)
            nc.scalar.activation(
                rows[:, 0:512], ssq[:], mybir.ActivationFunctionType.Sqrt,
                bias=beps_t[0:1, :], scale=float(1.0 / HD),
            )
            nc.scalar.activation(
                rows[:, 512:1024], ssq2[:], mybir.ActivationFunctionType.Sqrt,
                bias=beps_t[0:1, :], scale=float(1.0 / HD),
            )
            rq_rows[h] = rows
        else:
            kvh = ot
            ssqc = ps.tile([128, NB], f32, name=f"ssqk_{ci}_{kvh}", tag="misc", bufs=2)
            for b in range(NB):
                nc.tensor.matmul(
                    ssqc[:, b : b + 1],
                    lhsT=sqh[b // 4][:, (b % 4) * 128 : (b % 4 + 1) * 128],
                    rhs=onesb_t[:],
                )
            rootc = sb.tile([128, NB], f32, name=f"rootk_{ci}_{kvh}", tag="rk", bufs=10)
            nc.scalar.activation(
                rootc[:], ssqc[:], mybir.ActivationFunctionType.Sqrt,
                bias=beps_t[:, :], scale=float(1.0 / HD),
            )
            rkinv = sb.tile([128, NB], f32, name=f"rk_{ci}_{kvh}", tag="rk", bufs=10)
            nc.vector.reciprocal(rkinv[:], rootc[:])
            rk_cols[kvh] = rkinv

        # RoPE (all bf16): fin <- raw*cos + (R @ raw)*sin [*1/rms for q]
        fin = sb.tile([128, CHUNK], bf16, name=f"fin_{ci}_{ot}", tag="qb", bufs=11)
        for sh in range(2):
            rot = ps.tile([128, 512], f32, name=f"rot_{ci}_{ot}_{sh}", tag="mm", bufs=6)
            nc.tensor.matmul(
                rot[:], lhsT=rmat_t[:], rhs=raw[:, sh * 512 : (sh + 1) * 512]
            )
            nc.vector.tensor_tensor(
                rot[:], rot[:], sintab_t[:, s0 + sh * 512 : s0 + (sh + 1) * 512],
                mybir.AluOpType.mult,
            )
            nc.vector.tensor_tensor(
                raw[:, sh * 512 : (sh + 1) * 512],
                raw[:, sh * 512 : (sh + 1) * 512],
                costab_t[:, s0 + sh * 512 : s0 + (sh + 1) * 512],
                mybir.AluOpType.mult,
            )
            if is_q:
                nc.vector.tensor_tensor(
                    raw[:, sh * 512 : (sh + 1) * 512],
                    raw[:, sh * 512 : (sh + 1) * 512],
                    rot[:],
                    mybir.AluOpType.add,
                )
            else:
                nc.vector.tensor_tensor(
                    fin[:, sh * 512 : (sh + 1) * 512],
                    raw[:, sh * 512 : (sh + 1) * 512],
                    rot[:],
                    mybir.AluOpType.add,
                )
        if is_q:
            def fin_q(ot=ot, raw=raw, fin=fin):
                rows = rq_rows[ot - 4]
                for sh in range(2):
                    rep = ps.tile([128, 512], f32, name=f"repq_{ci}_{ot}_{sh}", tag="mm", bufs=6)
                    nc.tensor.matmul(
                        rep[:], lhsT=onesr_t[:],
                        rhs=rows[:, sh * 512 : (sh + 1) * 512],
                    )
                    rinv = sb.tile([128, 512], f32, name=f"rq_{ci}_{ot}_{sh}", tag="sq", bufs=4)
                    nc.vector.reciprocal_approx_fast(rinv[:], rep[:])
                    nc.vector.tensor_tensor(
                        fin[:, sh * 512 : (sh + 1) * 512],
                        raw[:, sh * 512 : (sh + 1) * 512],
                        rinv[:],
                        mybir.AluOpType.mult,
                    )
            pending.append(fin_q)
        fin_tiles[ot] = fin

    DPT = D // 128  # d-tiles
    xts = io["xts"].setdefault(ci, [None] * DPT)

    def xtile(dk):
        if xts[dk] is None:
            xt = sb.tile([128, CHUNK], SDT, name=f"xt_{ci}_{dk}", tag="xt",
                         bufs=DPT + XT_EXTRA)
            nc.sync.dma_start(
                xt[:],
                xT[dk * 128 : (dk + 1) * 128, s0 : s0 + CHUNK],
            )
            xts[dk] = xt
        return xts[dk]

    def prefetch_next():
        nxts = io["xts"].setdefault(ci + 1, [None] * DPT)
        ns0 = (ci + 1) * CHUNK
        for dk in range(8):
            xt = sb.tile([128, CHUNK], SDT, name=f"xt_{ci+1}_{dk}", tag="xt",
                         bufs=DPT + XT_EXTRA)
            nc.sync.dma_start(
                xt[:],
                xT[dk * 128 : (dk + 1) * 128, ns0 : ns0 + CHUNK],
            )
            nxts[dk] = xt
    io["prefetch"] = prefetch_next if ci + 1 < NCH else None

    post_q = []  # deferred post() work, flushed one og-group later

    def flush_posts():
        work = post_q[:]
        post_q.clear()
        flushq = pending[:]
        pending.clear()
        for ot in work:
            post(ot)
        for f in flushq:
            f()

    DKB = 8  # d-tiles per weight DMA (batched to offload the sync engine)
    for og in range(NOT // GW):
        accs = [
            ps.tile([128, 512], f32, name=f"pj_{ci}_{og}_{i}", tag="mm", bufs=6)
            for i in range(2 * GW)
        ]
        wts = [None] * (DPT // DKB)
        for dk in range(DPT):
            if dk % DKB == 0:
                wt = sb.tile([128, 128 * GW * DKB], SDT,
                             name=f"wt_{ci}_{og}_{dk}", tag="w", bufs=3)
                nc.sync.dma_start(
                    wt[:], wqkvT[og, dk // DKB], 
                )
                wts[dk // DKB] = wt
            wt = wts[dk // DKB]
            wo_ = (dk % DKB) * 128 * GW
            if dk == 2 and og > 0:
                # defer the previous group's post-processing until the PE has
                # fresh projection work queued, so PE never waits on it
                flush_posts()
            for j in range(GW):
                for sh in range(2):
                    nc.tensor.matmul(
                        accs[2 * j + sh][:],
                        lhsT=wt[:, wo_ + j * 128 : wo_ + (j + 1) * 128],
                        rhs=xtile(dk)[:, sh * 512 : (sh + 1) * 512],
                        start=(dk == 0),
                        stop=(dk == DPT - 1),
                    )
        for j in range(GW):
            ot = og * GW + j
            for sh in range(2):
                dst = qk_tiles[ot][:, sh * 512 : (sh + 1) * 512]
                if (2 * j + sh) % 2 == 0:
                    nc.scalar.copy(dst, accs[2 * j + sh][:])
                else:
                    nc.vector.tensor_copy(dst, accs[2 * j + sh][:])
            post_q.append(ot)
    flush_posts()

    # ---------------- Phase 2: attention (block-causal within chunk) ----------------
    # the last projection group's post-processing (flush_posts below) hides
    # under head 0's score matmuls instead of stalling the PE stream
    attn_tiles = []
    hpending = []
    rks_t = [None] * KH
    for kvh in range(KH):
        rks = sb.tile([128, NB], f32, name=f"rks_{ci}_{kvh}", tag="rk", bufs=10)
        nc.vector.tensor_scalar_mul(rks[:], rk_cols[kvh][:], float(ISQ_HD))
        rks_t[kvh] = rks
    for kvh in range(KH):
        kf = fin_tiles[kvh]
        vt = v_tok[kvh]
        rks = rks_t[kvh]
        for h4 in range(4):
            h = kvh * 4 + h4
            qf = fin_tiles[4 + h]
            pvA = ps.tile([128, 512], f32, name=f"pvA_{ci}_{h}", tag="mm", bufs=6)
            pvB = ps.tile([128, 512], f32, name=f"pvB_{ci}_{h}", tag="mm", bufs=6)
            smA = ps.tile([1, 512], f32, name=f"smA_{ci}_{h}", tag="misc", bufs=2)
            smB = ps.tile([1, 512], f32, name=f"smB_{ci}_{h}", tag="misc", bufs=2)
            flush = hpending[:]
            hpending.clear()
            pend1 = pending[:]
            pending.clear()
            for j in range(NB):
                w = CHUNK - j * 128
                lenA = 512 - j * 128 if j < 4 else 0
                pt = sb.tile([128, CHUNK], bf16, name=f"pt_{ci}_{h}_{j}", tag="pt", bufs=3)
                if lenA > 0:
                    sc = ps.tile([128, 512], f32, name=f"scA_{ci}_{h}_{j}", tag="mm", bufs=6)
                    nc.tensor.matmul(
                        sc[:, 0:lenA],
                        lhsT=kf[:, j * 128 : (j + 1) * 128],
                        rhs=qf[:, j * 128 : 512],
                    )
                    nc.scalar.activation(
                        pt[:, 0:lenA], sc[:, 0:lenA],
                        mybir.ActivationFunctionType.Exp,
                        scale=rks[:, j : j + 1],
                    )
                sc2 = ps.tile([128, 512], f32, name=f"scB_{ci}_{h}_{j}", tag="mm", bufs=6)
                b0 = max(512, j * 128)
                nc.tensor.matmul(
                    sc2[:, 0 : CHUNK - b0],
                    lhsT=kf[:, j * 128 : (j + 1) * 128],
                    rhs=qf[:, b0:CHUNK],
                )
                nc.scalar.activation(
                    pt[:, b0 - j * 128 : w], sc2[:, 0 : CHUNK - b0],
                    mybir.ActivationFunctionType.Exp,
                    scale=rks[:, j : j + 1],
                )
                nc.vector.tensor_tensor(
                    pt[:, 0:128], pt[:, 0:128], mask_t[:], mybir.AluOpType.mult
                )
                if lenA > 0:
                    nc.tensor.matmul(
                        smA[:, j * 128 : 512], lhsT=onesb_t[:], rhs=pt[:, 0:lenA],
                        start=(j == 0), stop=(j == 3),
                    )
                nc.tensor.matmul(
                    smB[:, b0 - 512 : 512], lhsT=onesb_t[:],
                    rhs=pt[:, b0 - j * 128 : w],
                    start=(j == 0), stop=(j == NB - 1),
                )
                if lenA > 0:
                    nc.tensor.matmul(
                        pvA[:, j * 128 : 512],
                        lhsT=vt[:, j * 128 : (j + 1) * 128],
                        rhs=pt[:, 0:lenA],
                        start=(j == 0), stop=(j == 3),
                    )
                nc.tensor.matmul(
                    pvB[:, b0 - 512 : 512],
                    lhsT=vt[:, j * 128 : (j + 1) * 128],
                    rhs=pt[:, b0 - j * 128 : w],
                    start=(j == 0), stop=(j == NB - 1),
                )
            for f in flush + pend1:
                f()
            # denominators: copy the sum rows out of PSUM (ACT Copy -> bf16),
            # replicate across partitions on PE, reciprocal on DVE; the
            # normalize is deferred one head so PE never stalls on this chain.
            srow = sb.tile([1, CHUNK], bf16, name=f"srow_{ci}_{h}", tag="rowq", bufs=3)
            nc.vector.tensor_copy(srow[:, 0:512], smA[:])
            nc.vector.tensor_copy(srow[:, 512:1024], smB[:])
            at = sb.tile([128, CHUNK], SDT, name=f"attn_{ci}_{h}", tag="attn", bufs=8)

            def fin_head(h=h, pvA=pvA, pvB=pvB, srow=srow, at=at):
                for sh, pv in ((0, pvA), (1, pvB)):
                    rep = ps.tile([128, 512], f32, name=f"reps_{ci}_{h}_{sh}", tag="mm", bufs=6)
                    nc.tensor.matmul(
                        rep[:], lhsT=onesr_t[:],
                        rhs=srow[:, sh * 512 : (sh + 1) * 512],
                    )
                    rinv = sb.tile([128, 512], f32, name=f"rinv_{ci}_{h}_{sh}", tag="sq", bufs=4)
                    nc.vector.reciprocal_approx_fast(rinv[:], rep[:])
                    nc.vector.tensor_tensor(
                        at[:, sh * 512 : (sh + 1) * 512],
                        pv[:],
                        rinv[:],
                        mybir.AluOpType.mult,
                    )
            hpending.append(fin_head)
            attn_tiles.append(at)

    for f in hpending:
        f()
    hpending.clear()

    # ---------------- Phase 3: output projection ----------------
    # y^T[dd, s] = sum_h woT[o_h, dd].T @ attn^T[o_h, s]; one 128-row dd tile
    # per group (2 PSUM banks) for deep cross-group pipelining.
    for dd in range(32):
        if dd == 16 and io.get("prefetch") is not None:
            io["prefetch"]()
        yps = [
            ps.tile([128, 512], f32, name=f"y_{ci}_{dd}_{i}", tag="mm", bufs=6)
            for i in range(2)
        ]
        wob = sb.tile([128, QO], SDT, name=f"wo_{ci}_{dd}", tag="wo", bufs=4)
        nc.sync.dma_start(wob[:], woT[dd])
        for h in range(QH):
            for sh in range(2):
                nc.tensor.matmul(
                    yps[sh][:],
                    lhsT=wob[:, h * 128 : (h + 1) * 128],
                    rhs=attn_tiles[h][:, sh * 512 : (sh + 1) * 512],
                    start=(h == 0), stop=(h == QH - 1),
                )
        ysb = sb.tile([128, CHUNK], bf16, name=f"ysb_{ci}_{dd}", tag="y", bufs=2)
        nc.vector.tensor_copy(ysb[:, 0:512], yps[0][:])
        nc.scalar.copy(ysb[:, 512:1024], yps[1][:])
        nc.sync.dma_start(
            yT[dd * 128 : (dd + 1) * 128, s0 : s0 + CHUNK], ysb[:]
        )


def _build_program():
    nc = bacc.Bacc(
        "TRN2",
        target_bir_lowering=False,
        debug=False,
        enable_asserts=False,
        num_devices=NCORES,
    )
    xT = nc.dram_tensor("xT", [D, TOK], SDT, kind="ExternalInput").ap()
    # batched-DMA layout: [og, dkb, p, (dk in batch) x (GW*128 cols)]
    wqkvT = nc.dram_tensor(
        "wqkvT", [QKVO // 256, 4, 128, 2048], SDT, kind="ExternalInput"
    ).ap()
    # per-dd-block tiled layout: woT[dd, p, h*128+c] = wo[dd*128+c, hg*QO + h*128+p]
    woT = nc.dram_tensor("woT", [D // 128, 128, QO], SDT, kind="ExternalInput").ap()
    costab = nc.dram_tensor("costab", [HD, TOK], f32, kind="ExternalInput").ap()
    sintab = nc.dram_tensor("sintab", [HD, TOK], f32, kind="ExternalInput").ap()
    rmat = nc.dram_tensor("rmat", [128, 128], bf16, kind="ExternalInput").ap()
    identb = nc.dram_tensor("identb", [128, 128], bf16, kind="ExternalInput").ap()
    mask = nc.dram_tensor("mask", [128, 128], bf16, kind="ExternalInput").ap()
    onesb = nc.dram_tensor("onesb", [128, 1], bf16, kind="ExternalInput").ap()
    beps = nc.dram_tensor("beps", [128, 1], f32, kind="ExternalInput").ap()
    onesr = nc.dram_tensor("onesr", [1, 128], bf16, kind="ExternalInput").ap()
    yT = nc.dram_tensor("yT", [D, TOK], bf16, kind="ExternalOutput").ap()

    with tile.TileContext(nc) as tc, ExitStack() as ctx:
        ctx.enter_context(nc.allow_low_precision(reason="bf16 attention operands"))
        sb = ctx.enter_context(tc.tile_pool(name="sb", bufs=1))
        ps = ctx.enter_context(tc.tile_pool(name="ps", bufs=1, space="PSUM"))
        cp = ctx.enter_context(tc.tile_pool(name="cp", bufs=1))

        consts = {}
        for nm, ap_, shape, dt_ in (
            ("costab", costab, [HD, TOK], f32),
            ("sintab", sintab, [HD, TOK], f32),
            ("rmat", rmat, [128, 128], bf16),
            ("identb", identb, [128, 128], bf16),
            ("mask", mask, [128, 128], bf16),
            ("onesb", onesb, [128, 1], bf16),
            ("beps", beps, [128, 1], f32),
            ("onesr", onesr, [1, 128], bf16),
        ):
            t = cp.tile(shape, dt_, name=f"c_{nm}")
            nc.gpsimd.dma_start(t[:], ap_[:])
            consts[nm] = t

        io = {"dram": (xT, wqkvT, woT, yT), "consts": consts, "xts": {}}
        pools = {"sb": sb, "ps": ps}
        for ci in range(NCH):
            _emit_chunk(ctx, tc, ci, io, pools)

    nc.compile()
    return nc


def _host_inputs(x, wq, wk, wv, wo):
    xf = np.ascontiguousarray(x.reshape(S, D).T.astype(SNP))  # [D, S]
    half = HD // 2
    inv_freq = (1.0 / (THETA ** (np.arange(0, half, dtype=np.float32) / half))).astype(
        np.float32
    )
    ang = np.arange(S, dtype=np.float32)[:, None] * inv_freq[None, :]
    cos = np.cos(ang).astype(np.float32)
    sin = np.sin(ang).astype(np.float32)
    costab = np.empty((HD, S), np.float32)
    sintab = np.empty((HD, S), np.float32)
    costab[0::2, :] = cos.T
    costab[1::2, :] = cos.T
    sintab[0::2, :] = sin.T
    sintab[1::2, :] = sin.T

    rmat = np.zeros((128, 128), ml_dtypes.bfloat16)
    for i in range(64):
        rmat[2 * i + 1, 2 * i] = -1.0
        rmat[2 * i, 2 * i + 1] = 1.0
    identb = np.eye(128, dtype=ml_dtypes.bfloat16)
    mask = np.triu(np.ones((128, 128), np.float32)).astype(ml_dtypes.bfloat16)
    onesb = np.ones((128, 1), ml_dtypes.bfloat16)
    beps = np.full((128, 1), EPS, np.float32)
    onesr = np.ones((1, 128), ml_dtypes.bfloat16)

    xT_cg = [np.ascontiguousarray(xf[:, cg * TOK : (cg + 1) * TOK]) for cg in range(CG)]
    cos_cg = [np.ascontiguousarray(costab[:, cg * TOK : (cg + 1) * TOK]) for cg in range(CG)]
    sin_cg = [np.ascontiguousarray(sintab[:, cg * TOK : (cg + 1) * TOK]) for cg in range(CG)]
    wqkvT_hg = []
    woT_hg = []
    for hg in range(HG):
        wq_c = wq[hg * QO : (hg + 1) * QO]
        wk_c = wk[hg * KO : (hg + 1) * KO]
        wv_c = wv[hg * KO : (hg + 1) * KO]
        # column order on device: [k, v, q]; batched-DMA tiling:
        # [og, dkb, p, dk_in_batch*256 + c] = W^T[dkb*1024 + dk*128 + p, og*256 + c]
        wflat = np.concatenate([wk_c, wv_c, wq_c], 0).T.astype(SNP)  # [D, QKVO]
        wt4 = wflat.reshape(4, 8, 128, QKVO // 256, 256)  # [dkb, dk, p, og, c]
        wqkvT_hg.append(np.ascontiguousarray(wt4.transpose(3, 0, 2, 1, 4).reshape(
            QKVO // 256, 4, 128, 2048)))
        wo_c = wo[:, hg * QO : (hg + 1) * QO]  # [D, QO]
        woH = wo_c.reshape(D // 128, 128, QH, 128).transpose(0, 3, 2, 1)  # [dd, p, hb, c]
        woT_hg.append(np.ascontiguousarray(woH.reshape(D // 128, 128, QO).astype(SNP)))

    in_maps = []
    for c in range(NCORES):
        cg, hg = c // HG, c % HG
        in_maps.append(
            {
                "xT": xT_cg[cg],
                "wqkvT": wqkvT_hg[hg],
                "woT": woT_hg[hg],
                "costab": cos_cg[cg],
                "sintab": sin_cg[cg],
                "rmat": rmat,
                "identb": identb,
                "mask": mask,
                "onesb": onesb,
                "beps": beps,
                "onesr": onesr,
            }
        )
    return in_maps


def _assemble(results):
    y = np.empty((S, D), np.float32)
    for cg in range(CG):
        acc = results[cg * HG]["yT"].astype(np.float32)
        for hg in range(1, HG):
            acc = acc + results[cg * HG + hg]["yT"].astype(np.float32)
        y[cg * TOK : (cg + 1) * TOK, :] = acc.T
    return y.reshape(1, S, D)


def kernel(x, wq, wk, wv, wo, **_kw):
    x = np.asarray(x, np.float32)
    wq = np.asarray(wq, np.float32)
    wk = np.asarray(wk, np.float32)
    wv = np.asarray(wv, np.float32)
    wo = np.asarray(wo, np.float32)

    if "nc" not in _CACHE:
        _CACHE["nc"] = _build_program()
    nc = _CACHE["nc"]
    in_maps = _host_inputs(x, wq, wk, wv, wo)
    res = run_bass_kernel_spmd(nc, in_maps, core_ids=list(range(NCORES)))
    _CACHE["last_result"] = res
    return _assemble(res.results)


def run_traced(x, wq, wk, wv, wo):
    """Like kernel() but with NTFF tracing; returns (out, BassKernelResults)."""
    if "nc" not in _CACHE:
        _CACHE["nc"] = _build_program()
    nc = _CACHE["nc"]
    in_maps = _host_inputs(
        np.asarray(x, np.float32), np.asarray(wq, np.float32),
        np.asarray(wk, np.float32), np.asarray(wv, np.float32),
        np.asarray(wo, np.float32),
    )
    res = run_bass_kernel_spmd(nc, in_maps, core_ids=list(range(NCORES)), trace=True)
    return _assemble(res.results), res


# revision 7
# speedup vs baseline: 1.1288x; 1.1184x over previous
"""Trainium2 Bass kernel for chunked-local GQA attention (nn_Attention_12266426597578).

Full-input contract: kernel(**inputs) takes the unsharded numpy inputs
(x [1,4096,4096], wq [4096,4096], wk [1024,4096], wv [1024,4096],
wo [4096,4096]) and returns the full output [1,4096,4096].

Sharding (8 cores): 2 chunk-groups (2 attention chunks of 1024 tokens each)
x 4 head-groups (8 q-heads / 2 kv-heads each). Each core computes a partial
y^T [D, 2048] for its token range; the host sums the 4 head-group partials
per token range and concatenates.

Device-side layout: everything runs in the "transposed" layout (feature dim
on SBUF partitions, tokens on the free axis) so that QKV projections,
RoPE (a 128x128 pair-rotation matmul), RMSNorm reductions (ones-vector
matmuls), scores^T, softmax denominators and PV all map onto the PE array
with no on-device transposition of x or the weights (the host pre-transposes
them instead).

v2 notes (trace-driven): the projection tiles live in bf16 end-to-end (no
fp32 shadow, no gpsimd cast on the RoPE critical path), squares run on the
vector engine, and every 1/x / 1/sqrt(x) row is computed with the DVE
reciprocal_approx_fast custom op on replicated tiles instead of the ACT
Ln/Exp table dance (ACT keeps only Exp + Sqrt + Copy; no per-head table
reloads). Post-processing of each projection group is deferred one group so
the PE stream stays dense and the HAM clock gate stays warm.
"""

import sys

sys.path.insert(0, "/opt/trn_rl_repo")

import numpy as np
import ml_dtypes
from contextlib import ExitStack

import concourse.bass as bass  # noqa: F401
import concourse.tile as tile
from concourse import bacc, mybir
from concourse.bass_utils import run_bass_kernel_spmd

# Problem constants (hardcoded per contract)
S, D = 4096, 4096
H, KVH, HD = 32, 8, 128
CHUNK = 1024
EPS = 1e-5
THETA = 500000.0
ISQ_HD = 1.0 / np.sqrt(np.float32(HD))

# Sharding
NCORES = 8
CG = 2  # chunk groups (token split)
HG = 4  # head groups
TOK = S // CG  # 2048 tokens per core
NCH = TOK // CHUNK  # 2 chunks per core
QH = H // HG  # 8 q heads per core
KH = KVH // HG  # 2 kv heads per core
QO = QH * HD  # 1024
KO = KH * HD  # 256
QKVO = QO + 2 * KO  # 1536
NOT = QKVO // 128  # 12 o-tiles: 0..1 k, 2..3 v, 4..11 q
NB = CHUNK // 128  # 8 blocks of 128 tokens per chunk

f32 = mybir.dt.float32
f32r = mybir.dt.float32r
bf16 = mybir.dt.bfloat16

# knobs
WBUFS = 8
XT_EXTRA = 2
GW = 2  # o-tiles per PSUM group in phase 1
_CACHE = {}

SDT = bf16
SNP = ml_dtypes.bfloat16


def _emit_chunk(ctx, tc, ci, io, pools):
    nc = tc.nc
    (xT, wqkvT, woT, yT) = io["dram"]
    consts = io["consts"]
    sb, ps = pools["sb"], pools["ps"]
    s0 = ci * CHUNK

    onesb_t = consts["onesb"]
    beps_t = consts["beps"]
    rmat_t = consts["rmat"]
    identb_t = consts["identb"]
    mask_t = consts["mask"]
    costab_t = consts["costab"]
    sintab_t = consts["sintab"]
    onesr_t = consts["onesr"]

    # ---------------- Phase 1: QKV projection (+RoPE +RMSNorm) ----------------
    # o-tile order (host-side wqkv concat order): k0 k1 v0 v1 q0..q7 so that
    # k/v post-processing and the first attention heads start while later q
    # projections are still running.
    qk_tiles = []
    for ot in range(NOT):
        t = sb.tile([128, CHUNK], bf16, name=f"qkv_{ci}_{ot}", tag="qk", bufs=6)
        qk_tiles.append(t)

    rq_rows = [None] * QH
    rk_cols = [None] * KH
    v_tok = [None] * KH
    fin_tiles = [None] * NOT
    pending = []

    def post(ot):
        """RoPE + RMSNorm (q/k) or transpose (v); all-bf16 tiles feed the
        attention matmuls. 1/sqrt rows use DVE reciprocal_approx_fast + ACT
        Sqrt (no Ln/Exp -> no ACT table thrash)."""
        raw = qk_tiles[ot]
        if 2 <= ot < 4:  # v: bf16 PE transpose per 128-block
            kvh = ot - 2
            vt = sb.tile([128, CHUNK], bf16, name=f"v_{ci}_{kvh}", tag="v", bufs=2)
            for b in range(NB):
                tp = ps.tile([128, 128], bf16, name=f"vtp_{ci}_{kvh}_{b}", tag="misc", bufs=2)
                nc.tensor.matmul(
                    tp[:],
                    lhsT=raw[:, b * 128 : (b + 1) * 128],
                    rhs=identb_t[:],
                    is_transpose=True,
                )
                nc.vector.tensor_copy(vt[:, b * 128 : (b + 1) * 128], tp[:])
            v_tok[kvh] = vt
            return

        is_q = ot >= 4
        # sumsq over head-dim partitions (RoPE-invariant -> use raw tile);
        # squares on DVE (bf16 2x mode), keeps Square off the ACT tables.
        sqh = []
        for sh in range(2):
            t = sb.tile([128, 512], bf16, name=f"sq_{ci}_{ot}_{sh}", tag="sq", bufs=4)
            nc.vector.tensor_tensor(
                t[:], raw[:, sh * 512 : (sh + 1) * 512],
                raw[:, sh * 512 : (sh + 1) * 512], mybir.AluOpType.mult,
            )
            sqh.append(t)
        if is_q:
            # row 1/rms: mean+eps via ACT Copy (scale/bias), reciprocal on the
            # row with the fast DVE approx, then ACT Sqrt -> bf16 row.
            h = ot - 4
            ssq = ps.tile([1, 512], f32, name=f"ssqa_{ci}_{ot}", tag="misc", bufs=2)
            ssq2 = ps.tile([1, 512], f32, name=f"ssqb_{ci}_{ot}", tag="misc", bufs=2)
            nc.tensor.matmul(ssq[:], lhsT=onesb_t[:], rhs=sqh[0][:])
            nc.tensor.matmul(ssq2[:], lhsT=onesb_t[:], rhs=sqh[1][:])
            mrow = sb.tile([1, CHUNK], f32, name=f"mq_{ci}_{ot}", tag="rowq", bufs=3)
            nc.scalar.activation(
                mrow[:, 0:512], ssq[:], mybir.ActivationFunctionType.Copy,
                bias=float(EPS), scale=float(1.0 / HD),
            )
            nc.scalar.activation(
                mrow[:, 512:1024], ssq2[:], mybir.ActivationFunctionType.Copy,
                bias=float(EPS), scale=float(1.0 / HD),
            )
            minv = sb.tile([1, CHUNK], f32, name=f"mi_{ci}_{ot}", tag="rowq", bufs=3)
            nc.vector.reciprocal_approx_fast(minv[:], mrow[:])
            rows = sb.tile([1, CHUNK], bf16, name=f"rqr_{ci}_{ot}", tag="rowq", bufs=3)
            nc.scalar.activation(rows[:], minv[:], mybir.ActivationFunctionType.Sqrt)
            rq_rows[h] = rows
        else:
            kvh = ot
            ssqc = ps.tile([128, NB], f32, name=f"ssqk_{ci}_{kvh}", tag="misc", bufs=2)
            for b in range(NB):
                nc.tensor.matmul(
                    ssqc[:, b : b + 1],
                    lhsT=sqh[b // 4][:, (b % 4) * 128 : (b % 4 + 1) * 128],
                    rhs=onesb_t[:],
                )
            rootc = sb.tile([128, NB], f32, name=f"rootk_{ci}_{kvh}", tag="rk", bufs=10)
            nc.scalar.activation(
                rootc[:], ssqc[:], mybir.ActivationFunctionType.Sqrt,
                bias=beps_t[:, :], scale=float(1.0 / HD),
            )
            rkinv = sb.tile([128, NB], f32, name=f"rk_{ci}_{kvh}", tag="rk", bufs=10)
            nc.vector.reciprocal(rkinv[:], rootc[:])
            rk_cols[kvh] = rkinv

        # RoPE (all bf16): fin <- raw*cos + (R @ raw)*sin [*1/rms for q]
        fin = sb.tile([128, CHUNK], bf16, name=f"fin_{ci}_{ot}", tag="qb", bufs=11)
        for sh in range(2):
            rot = ps.tile([128, 512], f32, name=f"rot_{ci}_{ot}_{sh}", tag="mm", bufs=6)
            nc.tensor.matmul(
                rot[:], lhsT=rmat_t[:], rhs=raw[:, sh * 512 : (sh + 1) * 512]
            )
            nc.vector.tensor_tensor(
                rot[:], rot[:], sintab_t[:, s0 + sh * 512 : s0 + (sh + 1) * 512],
                mybir.AluOpType.mult,
            )
            nc.vector.tensor_tensor(
                raw[:, sh * 512 : (sh + 1) * 512],
                raw[:, sh * 512 : (sh + 1) * 512],
                costab_t[:, s0 + sh * 512 : s0 + (sh + 1) * 512],
                mybir.AluOpType.mult,
            )
            if is_q:
                nc.vector.tensor_tensor(
                    raw[:, sh * 512 : (sh + 1) * 512],
                    raw[:, sh * 512 : (sh + 1) * 512],
                    rot[:],
                    mybir.AluOpType.add,
                )
            else:
                nc.vector.tensor_tensor(
                    fin[:, sh * 512 : (sh + 1) * 512],
                    raw[:, sh * 512 : (sh + 1) * 512],
                    rot[:],
                    mybir.AluOpType.add,
                )
        if is_q:
            def fin_q(ot=ot, raw=raw, fin=fin):
                rows = rq_rows[ot - 4]
                for sh in range(2):
                    rep = ps.tile([128, 512], f32, name=f"repq_{ci}_{ot}_{sh}", tag="mm", bufs=6)
                    nc.tensor.matmul(
                        rep[:], lhsT=onesr_t[:],
                        rhs=rows[:, sh * 512 : (sh + 1) * 512],
                    )
                    nc.vector.tensor_tensor(
                        fin[:, sh * 512 : (sh + 1) * 512],
                        raw[:, sh * 512 : (sh + 1) * 512],
                        rep[:],
                        mybir.AluOpType.mult,
                    )
            pending.append(fin_q)
        fin_tiles[ot] = fin

    DPT = D // 128  # d-tiles
    xts = io["xts"].setdefault(ci, [None] * DPT)

    def xtile(dk):
        if xts[dk] is None:
            xt = sb.tile([128, CHUNK], SDT, name=f"xt_{ci}_{dk}", tag="xt",
                         bufs=DPT + XT_EXTRA)
            nc.sync.dma_start(
                xt[:],
                xT[dk * 128 : (dk + 1) * 128, s0 : s0 + CHUNK],
            )
            xts[dk] = xt
        return xts[dk]

    def prefetch_next():
        nxts = io["xts"].setdefault(ci + 1, [None] * DPT)
        ns0 = (ci + 1) * CHUNK
        for dk in range(8):
            xt = sb.tile([128, CHUNK], SDT, name=f"xt_{ci+1}_{dk}", tag="xt",
                         bufs=DPT + XT_EXTRA)
            nc.sync.dma_start(
                xt[:],
                xT[dk * 128 : (dk + 1) * 128, ns0 : ns0 + CHUNK],
            )
            nxts[dk] = xt
    io["prefetch"] = prefetch_next if ci + 1 < NCH else None

    post_q = []  # deferred post() work, flushed one og-group later

    def flush_posts():
        work = post_q[:]
        post_q.clear()
        flushq = pending[:]
        pending.clear()
        for ot in work:
            post(ot)
        for f in flushq:
            f()

    DKB = 8  # d-tiles per weight DMA (batched to offload the sync engine)
    for og in range(NOT // GW):
        accs = [
            ps.tile([128, 512], f32, name=f"pj_{ci}_{og}_{i}", tag="mm", bufs=6)
            for i in range(2 * GW)
        ]
        wts = [None] * (DPT // DKB)
        for dk in range(DPT):
            if dk % DKB == 0:
                wt = sb.tile([128, 128 * GW * DKB], SDT,
                             name=f"wt_{ci}_{og}_{dk}", tag="w", bufs=3)
                nc.sync.dma_start(
                    wt[:], wqkvT[og, dk // DKB], 
                )
                wts[dk // DKB] = wt
            wt = wts[dk // DKB]
            wo_ = (dk % DKB) * 128 * GW
            if dk == 2 and og > 0:
                # defer the previous group's post-processing until the PE has
                # fresh projection work queued, so PE never waits on it
                flush_posts()
            for j in range(GW):
                for sh in range(2):
                    nc.tensor.matmul(
                        accs[2 * j + sh][:],
                        lhsT=wt[:, wo_ + j * 128 : wo_ + (j + 1) * 128],
                        rhs=xtile(dk)[:, sh * 512 : (sh + 1) * 512],
                        start=(dk == 0),
                        stop=(dk == DPT - 1),
                    )
        for j in range(GW):
            ot = og * GW + j
            for sh in range(2):
                dst = qk_tiles[ot][:, sh * 512 : (sh + 1) * 512]
                if (2 * j + sh) % 2 == 0:
                    nc.scalar.copy(dst, accs[2 * j + sh][:])
                else:
                    nc.vector.tensor_copy(dst, accs[2 * j + sh][:])
            post_q.append(ot)
    flush_posts()

    # ---------------- Phase 2: attention (block-causal within chunk) ----------------
    # software-pipelined heads: head h+1's score matmuls are emitted before
    # head h's sum/PV matmuls, so the PE always has independent work queued
    # while ACT runs head h's exps (keeps the HAM clock gate warm).
    attn_tiles = []
    hpending = []
    rks_t = [None] * KH
    for kvh in range(KH):
        rks = sb.tile([128, NB], f32, name=f"rks_{ci}_{kvh}", tag="rk", bufs=10)
        nc.vector.tensor_scalar_mul(rks[:], rk_cols[kvh][:], float(ISQ_HD))
        rks_t[kvh] = rks

    def stage_scores(h, kvh):
        """scores + exp + mask for all j of head h; returns the pt tiles."""
        kf = fin_tiles[kvh]
        qf = fin_tiles[4 + h]
        rks = rks_t[kvh]
        pts = []
        for j in range(NB):
            w = CHUNK - j * 128
            lenA = 512 - j * 128 if j < 4 else 0
            pt = sb.tile([128, w], bf16, name=f"pt_{ci}_{h}_{j}", tag=f"pt{j}", bufs=2)
            if lenA > 0:
                sc = ps.tile([128, 512], f32, name=f"scA_{ci}_{h}_{j}", tag="mm", bufs=6)
                nc.tensor.matmul(
                    sc[:, 0:lenA],
                    lhsT=kf[:, j * 128 : (j + 1) * 128],
                    rhs=qf[:, j * 128 : 512],
                )
                nc.scalar.activation(
                    pt[:, 0:lenA], sc[:, 0:lenA],
                    mybir.ActivationFunctionType.Exp,
                    scale=rks[:, j : j + 1],
                )
            sc2 = ps.tile([128, 512], f32, name=f"scB_{ci}_{h}_{j}", tag="mm", bufs=6)
            b0 = max(512, j * 128)
            nc.tensor.matmul(
                sc2[:, 0 : CHUNK - b0],
                lhsT=kf[:, j * 128 : (j + 1) * 128],
                rhs=qf[:, b0:CHUNK],
            )
            nc.scalar.activation(
                pt[:, b0 - j * 128 : w], sc2[:, 0 : CHUNK - b0],
                mybir.ActivationFunctionType.Exp,
                scale=rks[:, j : j + 1],
            )
            nc.vector.tensor_tensor(
                pt[:, 0:128], pt[:, 0:128], mask_t[:], mybir.AluOpType.mult
            )
            pts.append(pt)
        return pts

    def stage_pv(h, kvh, pts):
        """sums + PV for head h (exps already in flight from stage_scores)."""
        vt = v_tok[kvh]
        pvA = ps.tile([128, 512], f32, name=f"pvA_{ci}_{h}", tag="mm", bufs=6)
        pvB = ps.tile([128, 512], f32, name=f"pvB_{ci}_{h}", tag="mm", bufs=6)
        smA = ps.tile([1, 512], f32, name=f"smA_{ci}_{h}", tag="misc", bufs=2)
        smB = ps.tile([1, 512], f32, name=f"smB_{ci}_{h}", tag="misc", bufs=2)
        flush = hpending[:]
        hpending.clear()
        pend1 = pending[:]
        pending.clear()
        for j in range(NB):
            w = CHUNK - j * 128
            lenA = 512 - j * 128 if j < 4 else 0
            pt = pts[j]
            b0 = max(512, j * 128)
            if lenA > 0:
                nc.tensor.matmul(
                    smA[:, j * 128 : 512], lhsT=onesb_t[:], rhs=pt[:, 0:lenA],
                    start=(j == 0), stop=(j == 3),
                )
            nc.tensor.matmul(
                smB[:, b0 - 512 : 512], lhsT=onesb_t[:],
                rhs=pt[:, b0 - j * 128 : w],
                start=(j == 0), stop=(j == NB - 1),
            )
            if lenA > 0:
                nc.tensor.matmul(
                    pvA[:, j * 128 : 512],
                    lhsT=vt[:, j * 128 : (j + 1) * 128],
                    rhs=pt[:, 0:lenA],
                    start=(j == 0), stop=(j == 3),
                )
            nc.tensor.matmul(
                pvB[:, b0 - 512 : 512],
                lhsT=vt[:, j * 128 : (j + 1) * 128],
                rhs=pt[:, b0 - j * 128 : w],
                start=(j == 0), stop=(j == NB - 1),
            )
        for f in flush + pend1:
            f()
        # denominators: copy the sum rows out of PSUM, replicate across
        # partitions on PE, reciprocal on DVE; the normalize is deferred one
        # head so PE never stalls on this chain.
        srow = sb.tile([1, CHUNK], bf16, name=f"srow_{ci}_{h}", tag="rowq", bufs=3)
        nc.scalar.activation(srow[:, 0:512], smA[:], mybir.ActivationFunctionType.Copy)
        nc.scalar.activation(srow[:, 512:1024], smB[:], mybir.ActivationFunctionType.Copy)
        at = sb.tile([128, CHUNK], SDT, name=f"attn_{ci}_{h}", tag="attn", bufs=8)

        def fin_head(h=h, pvA=pvA, pvB=pvB, srow=srow, at=at):
            for sh, pv in ((0, pvA), (1, pvB)):
                rep = ps.tile([128, 512], f32, name=f"reps_{ci}_{h}_{sh}", tag="mm", bufs=6)
                nc.tensor.matmul(
                    rep[:], lhsT=onesr_t[:],
                    rhs=srow[:, sh * 512 : (sh + 1) * 512],
                )
                rinv = sb.tile([128, 512], f32, name=f"rinv_{ci}_{h}_{sh}", tag="sq", bufs=4)
                nc.vector.reciprocal_approx_fast(rinv[:], rep[:])
                nc.vector.tensor_tensor(
                    at[:, sh * 512 : (sh + 1) * 512],
                    pv[:],
                    rinv[:],
                    mybir.AluOpType.mult,
                )
        hpending.append(fin_head)
        attn_tiles.append(at)

    order = [(kvh * 4 + h4, kvh) for kvh in range(KH) for h4 in range(4)]
    prev = None
    for h, kvh in order:
        pts = stage_scores(h, kvh)
        if prev is not None:
            stage_pv(*prev)
        prev = (h, kvh, pts)
    stage_pv(*prev)

    for f in hpending:
        f()
    hpending.clear()

    # ---------------- Phase 3: output projection ----------------
    # y^T[dd, s] = sum_h woT[o_h, dd].T @ attn^T[o_h, s]; one 128-row dd tile
    # per group (2 PSUM banks) for deep cross-group pipelining.
    for dd in range(32):
        if dd == 16 and io.get("prefetch") is not None:
            io["prefetch"]()
        yps = [
            ps.tile([128, 512], f32, name=f"y_{ci}_{dd}_{i}", tag="mm", bufs=6)
            for i in range(2)
        ]
        wob = sb.tile([128, QO], SDT, name=f"wo_{ci}_{dd}", tag="wo", bufs=4)
        nc.sync.dma_start(wob[:], woT[dd])
        for h in range(QH):
            for sh in range(2):
                nc.tensor.matmul(
                    yps[sh][:],
                    lhsT=wob[:, h * 128 : (h + 1) * 128],
                    rhs=attn_tiles[h][:, sh * 512 : (sh + 1) * 512],
                    start=(h == 0), stop=(h == QH - 1),
                )
        ysb = sb.tile([128, CHUNK], bf16, name=f"ysb_{ci}_{dd}", tag="y", bufs=2)
        nc.vector.tensor_copy(ysb[:, 0:512], yps[0][:])
        nc.scalar.copy(ysb[:, 512:1024], yps[1][:])
        nc.sync.dma_start(
            yT[dd * 128 : (dd + 1) * 128, s0 : s0 + CHUNK], ysb[:]
        )


def _build_program():
    nc = bacc.Bacc(
        "TRN2",
        target_bir_lowering=False,
        debug=False,
        enable_asserts=False,
        num_devices=NCORES,
    )
    xT = nc.dram_tensor("xT", [D, TOK], SDT, kind="ExternalInput").ap()
    # batched-DMA layout: [og, dkb, p, (dk in batch) x (GW*128 cols)]
    wqkvT = nc.dram_tensor(
        "wqkvT", [QKVO // 256, 4, 128, 2048], SDT, kind="ExternalInput"
    ).ap()
    # per-dd-block tiled layout: woT[dd, p, h*128+c] = wo[dd*128+c, hg*QO + h*128+p]
    woT = nc.dram_tensor("woT", [D // 128, 128, QO], SDT, kind="ExternalInput").ap()
    costab = nc.dram_tensor("costab", [HD, TOK], f32, kind="ExternalInput").ap()
    sintab = nc.dram_tensor("sintab", [HD, TOK], f32, kind="ExternalInput").ap()
    rmat = nc.dram_tensor("rmat", [128, 128], bf16, kind="ExternalInput").ap()
    identb = nc.dram_tensor("identb", [128, 128], bf16, kind="ExternalInput").ap()
    mask = nc.dram_tensor("mask", [128, 128], bf16, kind="ExternalInput").ap()
    onesb = nc.dram_tensor("onesb", [128, 1], bf16, kind="ExternalInput").ap()
    beps = nc.dram_tensor("beps", [128, 1], f32, kind="ExternalInput").ap()
    onesr = nc.dram_tensor("onesr", [1, 128], bf16, kind="ExternalInput").ap()
    yT = nc.dram_tensor("yT", [D, TOK], bf16, kind="ExternalOutput").ap()

    with tile.TileContext(nc) as tc, ExitStack() as ctx:
        ctx.enter_context(nc.allow_low_precision(reason="bf16 attention operands"))
        sb = ctx.enter_context(tc.tile_pool(name="sb", bufs=1))
        ps = ctx.enter_context(tc.tile_pool(name="ps", bufs=1, space="PSUM"))
        cp = ctx.enter_context(tc.tile_pool(name="cp", bufs=1))

        consts = {}
        for nm, ap_, shape, dt_ in (
            ("costab", costab, [HD, TOK], f32),
            ("sintab", sintab, [HD, TOK], f32),
            ("rmat", rmat, [128, 128], bf16),
            ("identb", identb, [128, 128], bf16),
            ("mask", mask, [128, 128], bf16),
            ("onesb", onesb, [128, 1], bf16),
            ("beps", beps, [128, 1], f32),
            ("onesr", onesr, [1, 128], bf16),
        ):
            t = cp.tile(shape, dt_, name=f"c_{nm}")
            nc.gpsimd.dma_start(t[:], ap_[:])
            consts[nm] = t

        io = {"dram": (xT, wqkvT, woT, yT), "consts": consts, "xts": {}}
        pools = {"sb": sb, "ps": ps}
        for ci in range(NCH):
            _emit_chunk(ctx, tc, ci, io, pools)

    nc.compile()
    return nc


def _host_inputs(x, wq, wk, wv, wo):
    xf = np.ascontiguousarray(x.reshape(S, D).T.astype(SNP))  # [D, S]
    half = HD // 2
    inv_freq = (1.0 / (THETA ** (np.arange(0, half, dtype=np.float32) / half))).astype(
        np.float32
    )
    ang = np.arange(S, dtype=np.float32)[:, None] * inv_freq[None, :]
    cos = np.cos(ang).astype(np.float32)
    sin = np.sin(ang).astype(np.float32)
    costab = np.empty((HD, S), np.float32)
    sintab = np.empty((HD, S), np.float32)
    costab[0::2, :] = cos.T
    costab[1::2, :] = cos.T
    sintab[0::2, :] = sin.T
    sintab[1::2, :] = sin.T

    rmat = np.zeros((128, 128), ml_dtypes.bfloat16)
    for i in range(64):
        rmat[2 * i + 1, 2 * i] = -1.0
        rmat[2 * i, 2 * i + 1] = 1.0
    identb = np.eye(128, dtype=ml_dtypes.bfloat16)
    mask = np.triu(np.ones((128, 128), np.float32)).astype(ml_dtypes.bfloat16)
    onesb = np.ones((128, 1), ml_dtypes.bfloat16)
    beps = np.full((128, 1), EPS, np.float32)
    onesr = np.ones((1, 128), ml_dtypes.bfloat16)

    xT_cg = [np.ascontiguousarray(xf[:, cg * TOK : (cg + 1) * TOK]) for cg in range(CG)]
    cos_cg = [np.ascontiguousarray(costab[:, cg * TOK : (cg + 1) * TOK]) for cg in range(CG)]
    sin_cg = [np.ascontiguousarray(sintab[:, cg * TOK : (cg + 1) * TOK]) for cg in range(CG)]
    wqkvT_hg = []
    woT_hg = []
    for hg in range(HG):
        wq_c = wq[hg * QO : (hg + 1) * QO]
        wk_c = wk[hg * KO : (hg + 1) * KO]
        wv_c = wv[hg * KO : (hg + 1) * KO]
        # column order on device: [k, v, q]; batched-DMA tiling:
        # [og, dkb, p, dk_in_batch*256 + c] = W^T[dkb*1024 + dk*128 + p, og*256 + c]
        wflat = np.concatenate([wk_c, wv_c, wq_c], 0).T.astype(SNP)  # [D, QKVO]
        wt4 = wflat.reshape(4, 8, 128, QKVO // 256, 256)  # [dkb, dk, p, og, c]
        wqkvT_hg.append(np.ascontiguousarray(wt4.transpose(3, 0, 2, 1, 4).reshape(
            QKVO // 256, 4, 128, 2048)))
        wo_c = wo[:, hg * QO : (hg + 1) * QO]  # [D, QO]
        woH = wo_c.reshape(D // 128, 128, QH, 128).transpose(0, 3, 2, 1)  # [dd, p, hb, c]
        woT_hg.append(np.ascontiguousarray(woH.reshape(D // 128, 128, QO).astype(SNP)))

    in_maps = []
    for c in range(NCORES):
        cg, hg = c // HG, c % HG
        in_maps.append(
            {
                "xT": xT_cg[cg],
                "wqkvT": wqkvT_hg[hg],
                "woT": woT_hg[hg],
                "costab": cos_cg[cg],
                "sintab": sin_cg[cg],
                "rmat": rmat,
                "identb": identb,
                "mask": mask,
                "onesb": onesb,
                "beps": beps,
                "onesr": onesr,
            }
        )
    return in_maps


def _assemble(results):
    y = np.empty((S, D), np.float32)
    for cg in range(CG):
        acc = results[cg * HG]["yT"].astype(np.float32)
        for hg in range(1, HG):
            acc = acc + results[cg * HG + hg]["yT"].astype(np.float32)
        y[cg * TOK : (cg + 1) * TOK, :] = acc.T
    return y.reshape(1, S, D)


def kernel(x, wq, wk, wv, wo, **_kw):
    x = np.asarray(x, np.float32)
    wq = np.asarray(wq, np.float32)
    wk = np.asarray(wk, np.float32)
    wv = np.asarray(wv, np.float32)
    wo = np.asarray(wo, np.float32)

    if "nc" not in _CACHE:
        _CACHE["nc"] = _build_program()
    nc = _CACHE["nc"]
    in_maps = _host_inputs(x, wq, wk, wv, wo)
    res = run_bass_kernel_spmd(nc, in_maps, core_ids=list(range(NCORES)))
    _CACHE["last_result"] = res
    return _assemble(res.results)


def run_traced(x, wq, wk, wv, wo):
    """Like kernel() but with NTFF tracing; returns (out, BassKernelResults)."""
    if "nc" not in _CACHE:
        _CACHE["nc"] = _build_program()
    nc = _CACHE["nc"]
    in_maps = _host_inputs(
        np.asarray(x, np.float32), np.asarray(wq, np.float32),
        np.asarray(wk, np.float32), np.asarray(wv, np.float32),
        np.asarray(wo, np.float32),
    )
    res = run_bass_kernel_spmd(nc, in_maps, core_ids=list(range(NCORES)), trace=True)
    return _assemble(res.results), res
